# revision 1
# baseline (speedup 1.0000x reference)
"""Trainium2 Bass kernel for nn_DeformableTransposedConv.

Pipeline (per the reference):
  up  = ConvTranspose2d(x, trans_w, stride=2, pad=1, outpad=1)   # [N,128,128,128]
  off = tanh(conv(relu(conv(lateral_feat, w1)), w2))             # [N,18,1,1] -> broadcast
  out = deform_conv2d(up, off, trans_w, pad=1)                   # [N,256,128,128]

Key structure exploited:
  * The offsets are constant over space (1x1 lateral input broadcast), so the
    bilinear deformable gather collapses to a per-batch 5x5 conv with
    "effective" weights W_eff[n] built host-side from trans_w and the (tiny)
    offsets.  The device computes:
        out[n] = sum_{dy,dx in 5x5} W_eff[n,dy,dx] @ shift(up[n], dy, dx)
    as PSUM-accumulated matmuls over the 128 up-channels.
  * The stride-2 transposed conv splits into 4 phase sub-convs with
    {1,2,2,4} taps, each a PSUM-accumulated matmul over the 256 x-channels.

Sharding: 8 cores = 2 batches x 4 row-strips of 32 output rows.  Each core
computes out[n, :, 32r:32r+32, :] from a 20-row slice of x (with halo).
All weights / layout prep / zero padding is done host-side; the NEFF is
input-independent (weights and data are ExternalInputs).
"""

import numpy as np
import ml_dtypes

import concourse.bass as bass
import concourse.tile as tile
from concourse import bacc, mybir
from concourse.bass_utils import run_bass_kernel_spmd

BF16 = ml_dtypes.bfloat16

# ---- problem constants (hardcoded per contract) ----
N_BATCH = 2
CIN = 256
COUT = 128          # up channels
K = 3
PAD = 1
H0 = W0 = 64        # x spatial
H = W = 128         # up / out spatial
N_CORES = 8
STRIPS = 4          # row strips per batch
OUT_R = 32          # output rows per strip

# SBUF layout constants
XR, XC = 20, 66     # x tile rows (16 + 2 halo each side), cols (64 + 1 pad + 1 align)
UR, UC = 36, 132    # up tile rows (32 + 2 halo each side), cols (128 + 2 + 2)
NCELL = 25          # 5x5 effective deform kernel
RBLK = 4            # output rows per stage-B block (4*128 = 512 = one PSUM bank)

# stage-B variant:
#   "full25" = static 5x5 effective conv (25 matmul terms / block)
#   "slots"  = dynamic cell slots (pruned zero cells, runtime offsets)
#   "hybrid" = DVE bilinear blends + matmuls (y-blend on DVE for all taps;
#              x-blend on DVE for the first HYBRID_S taps, folded into scaled
#              weights for the rest)
import os as _os
VARIANT = _os.environ.get("KERNEL_VARIANT", "fp8r")
HYBRID_S = int(_os.environ.get("HYBRID_S", "4"))
SBR = 8             # hybrid blend superblock rows (2 PSUM blocks)

_CACHED_NC = {}


# --------------------------------------------------------------------------
# host-side preparation
# --------------------------------------------------------------------------

def _offsets_from_inputs(lateral_feat, off_w1, off_b1, off_w2, off_b2):
    """Tiny offset MLP (conv on 1x1 spatial input == center-tap matmul)."""
    lf = lateral_feat[:, :, 0, 0].astype(np.float32)                    # [N,128]
    h = np.maximum(0.0, lf @ off_w1[:, :, 1, 1].T.astype(np.float32)
                   + off_b1.astype(np.float32))                         # [N,64]
    off = np.tanh(h @ off_w2[:, :, 1, 1].T.astype(np.float32)
                  + off_b2.astype(np.float32)).astype(np.float32)       # [N,18]
    oy = off.reshape(-1, K * K, 2)[:, :, 0]
    ox = off.reshape(-1, K * K, 2)[:, :, 1]
    return oy, ox


def _w_eff(trans_w, oy, ox):
    """Effective 5x5 deform weights. Returns [N, 5, 5, 256(o), 128(c)] f32."""
    n_b = oy.shape[0]
    Weff = np.zeros((n_b, 5, 5, CIN, COUT), np.float32)
    for n in range(n_b):
        for k in range(K * K):
            ky, kx = k // K, k % K
            ay = np.float32(ky - 1) + oy[n, k]
            ax = np.float32(kx - 1) + ox[n, k]
            Ay, Ax = int(np.floor(ay)), int(np.floor(ax))
            dy = float(ay) - Ay
            dx = float(ax) - Ax
            tap = trans_w[:, :, ky, kx].astype(np.float32)
            for cy, wy in ((0, 1.0 - dy), (1, dy)):
                for cx, wx in ((0, 1.0 - dx), (1, dx)):
                    w = wy * wx
                    if w != 0.0:
                        Weff[n, Ay + cy + 2, Ax + cx + 2] += w * tap
    return Weff


def _prep_in_maps(x, trans_w, oy, ox):
    """Build the per-core input dicts (already bf16, padded, SBUF-layouts).
    Returns (in_maps, ncell) where ncell is the stage-B slot count."""
    xf = x.astype(np.float32)

    # stage-A weights, shared by all cores: wa[k, h2, j, m]
    wa = np.zeros((COUT, 2, 9, COUT), np.float32)
    for h2 in range(2):
        for j in range(9):
            jy, jx = j // 3, j % 3
            # lhsT[K=cin(128), M=cout(128)] = trans_w[h2*128+kk, m, jy, jx]
            wa[:, h2, j, :] = trans_w[h2 * 128:(h2 + 1) * 128, :, jy, jx]
    wa_b = wa.astype(BF16).reshape(COUT, 2 * 9 * COUT)

    if VARIANT == "hybrid":
        return _prep_in_maps_hybrid(xf, trans_w, oy, ox, wa_b)
    if VARIANT == "fp8r":
        return _prep_in_maps_fp8r(xf, trans_w, oy, ox, wa_b)

    # stage-B weights per batch
    Weff = _w_eff(trans_w, oy, ox)                      # [N,5,5,256,128]
    wb_all, co_all = [], []
    if VARIANT == "full25":
        ncell = NCELL
        for n in range(N_BATCH):
            wb = Weff[n].reshape(NCELL, 2, COUT, COUT)   # [cell, half, o(128), c]
            wb = wb.transpose(3, 0, 1, 2)                # [c, cell, half, o]
            wb_all.append(np.ascontiguousarray(wb).astype(BF16)
                          .reshape(COUT, NCELL * 2 * COUT))
            co_all.append(None)
    elif VARIANT == "union":
        # static program specialized on the union of nonzero cells across
        # batches (compile cache keyed on the union tuple)
        nz = [np.nonzero(np.abs(Weff[n]).reshape(25, -1).max(1) > 0)[0]
              for n in range(N_BATCH)]
        union = sorted(set(int(c) for z in nz for c in z))
        ncell = ("union",) + tuple(union)
        for n in range(N_BATCH):
            wb = np.zeros((len(union), 2, COUT, COUT), np.float32)
            for s, ci in enumerate(union):
                wb[s] = Weff[n, ci // 5, ci % 5].reshape(2, COUT, COUT)
            wb = wb.transpose(3, 0, 1, 2)
            wb_all.append(np.ascontiguousarray(wb).astype(BF16)
                          .reshape(COUT, len(union) * 2 * COUT))
            co_all.append(None)
    else:  # "slots": pruned nonzero cells, offsets shipped as data
        nz = [np.nonzero(np.abs(Weff[n]).reshape(25, -1).max(1) > 0)[0]
              for n in range(N_BATCH)]
        ncell = max(len(z) for z in nz)
        for n in range(N_BATCH):
            cells = list(nz[n]) + [12] * (ncell - len(nz[n]))  # pad w/ center
            wb = np.zeros((ncell, 2, COUT, COUT), np.float32)
            co = np.zeros((1, ncell, 2), np.int32)
            for s, ci in enumerate(cells):
                dyi, dxi = ci // 5, ci % 5
                if s < len(nz[n]):
                    wb[s] = Weff[n, dyi, dxi].reshape(2, COUT, COUT)
                co[0, s] = (dyi, dxi)
            wb = wb.transpose(3, 0, 1, 2)                # [c, slot, half, o]
            wb_all.append(np.ascontiguousarray(wb).astype(BF16)
                          .reshape(COUT, ncell * 2 * COUT))
            co_all.append(co)

    in_maps = []
    for core in range(N_CORES):
        n, r = core // STRIPS, core % STRIPS
        # x slice with halo: global x rows [16r-2, 16r+18)
        xs = np.zeros((COUT, 2, XR, XC), np.float32)
        r0 = 16 * r - 2
        lo, hi = max(0, r0), min(H0, r0 + XR)
        for h2 in range(2):
            xs[:, h2, lo - r0:hi - r0, :W0] = xf[n, h2 * 128:(h2 + 1) * 128, lo:hi, :]
        # bottom-halo validity mask: strip 0 must zero up rows g=-2,-1 which
        # the phase formula would otherwise fill with spurious values
        mk = np.full((COUT, 1), 0.0 if r == 0 else 1.0, np.float32)
        im = {
            "xs": np.ascontiguousarray(xs.astype(BF16).reshape(COUT, 2 * XR * XC)),
            "wa": wa_b,
            "wb": wb_all[n],
            "mk": mk,
        }
        if co_all[n] is not None:
            im["co"] = co_all[n]
        in_maps.append(im)
    return in_maps, ncell


FP8 = ml_dtypes.float8_e4m3
RING_SCALE = 256.0


def _prep_in_maps_fp8r(xf, trans_w, oy, ox, wa_b):
    """Union cells; big cells in bf16, small 'ring' cells paired into fp8
    DoubleRow matmuls (weights scaled by RING_SCALE)."""
    Weff = _w_eff(trans_w, oy, ox)                       # [N,5,5,256,128]
    norms = np.abs(Weff).reshape(N_BATCH, 25, -1).max(2)  # [N,25]
    union = sorted(set(np.nonzero(norms.max(0) > 0)[0].tolist()))
    thr = 0.25 * norms.max()
    bigs = [c for c in union if norms[:, c].max() > thr]
    rings = [c for c in union if c not in bigs]
    if len(rings) % 2:
        bigs.append(rings.pop())                          # odd leftover -> bf16
    # order by window offset (dx major, dy minor); pair far-apart cells so the
    # two DoubleRow K-group windows never overlap (overlapping windows were
    # measured ~1.7x slower on the PE)
    rings.sort(key=lambda c: (c % 5, c // 5))
    nh = len(rings) // 2
    pairs = [(rings[i], rings[i + nh]) for i in range(nh)]
    bigs = sorted(bigs)

    wb_all, wr_all = [], []
    for n in range(N_BATCH):
        wb = np.zeros((max(len(bigs), 1), 2, COUT, COUT), np.float32)
        for s, ci in enumerate(bigs):
            wb[s] = Weff[n, ci // 5, ci % 5].reshape(2, COUT, COUT)
        wb = wb.transpose(3, 0, 1, 2)                     # [c, slot, half, o]
        wb_all.append(np.ascontiguousarray(wb).astype(BF16)
                      .reshape(COUT, -1))
        wr = np.zeros((max(len(pairs), 1), 2, 2, COUT, COUT), np.float32)
        for p, (c1, c2) in enumerate(pairs):
            for half in range(2):
                wr[p, half, 0] = RING_SCALE * \
                    Weff[n, c1 // 5, c1 % 5][128 * half:128 * (half + 1)].T
                wr[p, half, 1] = RING_SCALE * \
                    Weff[n, c2 // 5, c2 % 5][128 * half:128 * (half + 1)].T
        # wr[p, half, ksub, c, o] -> [c, p, half, ksub, o]
        wr = wr.transpose(3, 0, 1, 2, 4)
        wr_all.append(np.ascontiguousarray(wr).astype(FP8).reshape(COUT, -1))

    in_maps = []
    for core in range(N_CORES):
        n, r = core // STRIPS, core % STRIPS
        xs = np.zeros((COUT, 2, XR, XC), np.float32)
        r0 = 16 * r - 2
        lo, hi = max(0, r0), min(H0, r0 + XR)
        for h2 in range(2):
            xs[:, h2, lo - r0:hi - r0, :W0] = xf[n, h2 * 128:(h2 + 1) * 128, lo:hi, :]
        mk = np.full((COUT, 1), 0.0 if r == 0 else 1.0, np.float32)
        in_maps.append({
            "xs": np.ascontiguousarray(xs.astype(BF16).reshape(COUT, 2 * XR * XC)),
            "wa": wa_b,
            "wb": wb_all[n],
            "wr": wr_all[n],
            "mk": mk,
        })
    return in_maps, ("fp8r", tuple(bigs), tuple(pairs))


def _prep_in_maps_hybrid(xf, trans_w, oy, ox, wa_b):
    S = HYBRID_S
    nslot = S + 2 * (9 - S)
    wb_all, bs_all, dsc_all, ofs_all = [], [], [], []
    for n in range(N_BATCH):
        wb = np.zeros((nslot, 2, COUT, COUT), np.float32)   # [slot, half, c, o]
        bs = np.zeros((9, 2), np.float32)
        dsc = np.zeros((max(S, 1), 2), np.float32)
        ofs = np.zeros((1, 9, 2), np.int32)
        for k in range(9):
            ky, kx = k // 3, k % 3
            ay = np.float32(ky - 1) + oy[n, k]
            ax = np.float32(kx - 1) + ox[n, k]
            Ay, Ax = int(np.floor(ay)), int(np.floor(ax))
            dy = float(ay) - Ay
            dx = float(ax) - Ax
            ofs[0, k] = (2 + Ay, 2 + Ax)
            bs[k] = (1.0 - dy, dy)
            wkT = np.stack([trans_w[h * 128:(h + 1) * 128, :, ky, kx].T
                            for h in range(2)])             # [half, c, o]
            if k < S:
                dsc[k] = (1.0 - dx, dx)
                wb[k] = wkT
            else:
                wb[S + 2 * (k - S) + 0] = (1.0 - dx) * wkT
                wb[S + 2 * (k - S) + 1] = dx * wkT
        wb = wb.transpose(2, 0, 1, 3)                       # [c, slot, half, o]
        wb_all.append(np.ascontiguousarray(wb).astype(BF16)
                      .reshape(COUT, nslot * 2 * COUT))
        bs_all.append(np.broadcast_to(bs.reshape(1, 9, 2),
                                      (COUT, 9, 2)).copy())
        dsc_all.append(np.broadcast_to(dsc.reshape(1, -1, 2),
                                       (COUT, max(S, 1), 2)).copy())
        ofs_all.append(ofs)

    in_maps = []
    for core in range(N_CORES):
        n, r = core // STRIPS, core % STRIPS
        xs = np.zeros((COUT, 2, XR, XC), np.float32)
        r0 = 16 * r - 2
        lo, hi = max(0, r0), min(H0, r0 + XR)
        for h2 in range(2):
            xs[:, h2, lo - r0:hi - r0, :W0] = xf[n, h2 * 128:(h2 + 1) * 128, lo:hi, :]
        mk = np.full((COUT, 1), 0.0 if r == 0 else 1.0, np.float32)
        in_maps.append({
            "xs": np.ascontiguousarray(xs.astype(BF16).reshape(COUT, 2 * XR * XC)),
            "wa": wa_b,
            "wb": wb_all[n],
            "mk": mk,
            "bs": bs_all[n].reshape(COUT, 18),
            "dsc": dsc_all[n].reshape(COUT, -1),
            "co": ofs_all[n],
        })
    return in_maps, nslot


# --------------------------------------------------------------------------
# device program (input-independent; same for all cores except r-dependent
# row validity -> handled by *uniform* structure: we compute all 36 up rows,
# rows outside [0,128) stay zero because their x inputs are zeroed host-side
# ... except parity bookkeeping differs per strip; we keep the program truly
# SPMD by computing the full 18 a'-rows per phase and masking via zero x.)
# --------------------------------------------------------------------------

def _build_nc(ncell):
    fp8r = isinstance(ncell, tuple) and ncell[0] == "fp8r"
    if fp8r:
        bigs, pairs = list(ncell[1]), list(ncell[2])
        ncell = max(len(bigs), 1)
        cells, dyn = None, False
    elif isinstance(ncell, tuple):      # ("union", cell, cell, ...)
        cells = list(ncell[1:])
        ncell = len(cells)
        dyn = False
    else:
        cells = list(range(NCELL)) if VARIANT == "full25" else None
        dyn = VARIANT not in ("full25",)
    nc = bacc.Bacc("TRN2", target_bir_lowering=False, debug=False,
                   enable_asserts=False)

    xs_d = nc.dram_tensor("xs", [COUT, 2 * XR * XC], mybir.dt.bfloat16,
                          kind="ExternalInput").ap()
    wa_d = nc.dram_tensor("wa", [COUT, 2 * 9 * COUT], mybir.dt.bfloat16,
                          kind="ExternalInput").ap()
    wb_d = nc.dram_tensor("wb", [COUT, ncell * 2 * COUT], mybir.dt.bfloat16,
                          kind="ExternalInput").ap()
    mk_d = nc.dram_tensor("mk", [COUT, 1], mybir.dt.float32,
                          kind="ExternalInput").ap()
    if fp8r:
        wr_d = nc.dram_tensor(
            "wr", [COUT, max(len(pairs), 1) * 2 * 2 * COUT],
            mybir.dt.float8e4, kind="ExternalInput").ap()
    hyb = VARIANT == "hybrid"
    S = HYBRID_S
    if hyb:
        co_d = nc.dram_tensor("co", [1, 9, 2], mybir.dt.int32,
                              kind="ExternalInput").ap()
        bs_d = nc.dram_tensor("bs", [COUT, 18], mybir.dt.float32,
                              kind="ExternalInput").ap()
        dsc_d = nc.dram_tensor("dsc", [COUT, 2 * max(S, 1)], mybir.dt.float32,
                               kind="ExternalInput").ap()
    elif dyn:
        co_d = nc.dram_tensor("co", [1, ncell, 2], mybir.dt.int32,
                              kind="ExternalInput").ap()
    out_d = nc.dram_tensor("out", [CIN, OUT_R, W], mybir.dt.float32,
                           kind="ExternalOutput").ap()

    with tile.TileContext(nc) as tc:
        with (
            tc.tile_pool(name="singles", bufs=1) as singles,
            tc.tile_pool(name="outp", bufs=4) as outp,
            tc.tile_pool(name="psB", bufs=4, space="PSUM") as psB,
            tc.tile_pool(name="psR", bufs=4, space="PSUM") as psR,
        ):
            xs_t = singles.tile([COUT, 2, XR, XC], mybir.dt.bfloat16)
            wa_t = singles.tile([COUT, 2, 9, COUT], mybir.dt.bfloat16)
            wb_t = singles.tile([COUT, ncell, 2, COUT], mybir.dt.bfloat16)
            mk_t = singles.tile([COUT, 1], mybir.dt.float32)
            # +12 pad: hybrid vy reads may run a few elements past the last
            # row (col-window spill); padded region is zeroed, never consumed
            up_full = singles.tile([COUT, UR * UC + 12], mybir.dt.bfloat16)
            up_t = up_full[:, :UR * UC]

            # stage-A critical inputs split across both HWDGE queues; xs is
            # further split by row band so the first stage-A band can start
            # after ~0.3MB instead of the whole tensor.  Band a0 reads xs rows
            # a0+1+dy (dy<=1), so rows [0,9) cover band 0, [9,20) the rest.
            xs4 = xs_t[:]
            xs4_d = xs_d.rearrange("p (a b c) -> p a b c", a=2, b=XR)
            for h2 in range(2):
                eng = nc.sync if h2 == 0 else nc.scalar
                eng.dma_start(out=xs4[:, h2, 0:9, :], in_=xs4_d[:, h2, 0:9, :])
            nc.sync.dma_start(out=wa_t[:].rearrange("p a b c -> p (a b c)"), in_=wa_d)
            for h2 in range(2):
                eng = nc.scalar if h2 == 0 else nc.sync
                eng.dma_start(out=xs4[:, h2, 9:, :], in_=xs4_d[:, h2, 9:, :])
            nc.sync.dma_start(out=mk_t[:], in_=mk_d)
            wb_flat = wb_t[:].rearrange("p a b c -> p (a b c)")
            nc.scalar.dma_start(out=wb_flat, in_=wb_d)
            if fp8r:
                wr_t = singles.tile([COUT, max(len(pairs), 1), 2, 2, COUT],
                                    mybir.dt.float8e4)
                nc.sync.dma_start(
                    out=wr_t[:].rearrange("p a b c d -> p (a b c d)"), in_=wr_d)
                upf_t = singles.tile([COUT, 5, UR, W], mybir.dt.float8e4)
            if hyb:
                co_t = singles.tile([1, 9, 2], mybir.dt.int32)
                bs_t = singles.tile([COUT, 9, 2], mybir.dt.float32)
                dsc_t = singles.tile([COUT, max(S, 1), 2], mybir.dt.float32)
                nc.sync.dma_start(out=co_t[:].rearrange("p a b -> p (a b)"),
                                  in_=co_d.rearrange("p a b -> p (a b)"))
                nc.sync.dma_start(out=bs_t[:].rearrange("p a b -> p (a b)"),
                                  in_=bs_d)
                nc.sync.dma_start(out=dsc_t[:].rearrange("p a b -> p (a b)"),
                                  in_=dsc_d)
            elif dyn:
                co_t = singles.tile([1, ncell, 2], mybir.dt.int32)
                nc.sync.dma_start(out=co_t[:].rearrange("p a b -> p (a b)"),
                                  in_=co_d.rearrange("p a b -> p (a b)"))

            # zero the up tile (margins + potentially-invalid rows)
            nc.vector.memset(up_full[:], 0.0)

            # views of up: [p, a'(18), q(2), cc(66), r(2)] for phase writes,
            # [p, l(36), c(132)] for stage-B reads
            up_w = up_t.rearrange("p (a q c r) -> p a q c r", q=2, c=66, r=2)
            up_r = up_t.rearrange("p (l c) -> p l c", c=132)

            # ---- stage A: transposed conv -> up ----
            # row-major (a0 outer) so each 12-row band of up completes early;
            # for fp8r the band's fp8 casts are emitted right behind it, so
            # the ring matmuls never wait on a late cast burst
            ytaps = {0: ((1, 0),), 1: ((2, 0), (0, 1))}
            if fp8r:
                need_dx = sorted({c % 5 for pr in pairs for c in pr})
            for a0 in range(0, 18, 6):
                rc = 6
                for py in (0, 1):
                    for px in (0, 1):
                        taps = [(jy, dy, jx, dx)
                                for jy, dy in ytaps[py] for jx, dx in ytaps[px]]
                        # stage A borrows the ring pool (idle here) so its
                        # evacuations never block stage-B big-cell psum slots
                        pool = psR if fp8r else psB
                        ps = pool.tile([COUT, 6, 64], mybir.dt.float32,
                                       tag="psR" if fp8r else "psB")
                        nmm = len(taps) * 2
                        i = 0
                        for (jy, dy, jx, dx) in taps:
                            for h2 in range(2):
                                nc.tensor.matmul(
                                    ps[:, :rc, :],
                                    lhsT=wa_t[:, h2, jy * 3 + jx, :],
                                    rhs=xs_t[:, h2, a0 + 1 + dy:a0 + 1 + dy + rc,
                                             dx:dx + 64],
                                    start=(i == 0), stop=(i == nmm - 1),
                                )
                                i += 1
                        # scatter phase result into up (cast to bf16)
                        nc.scalar.copy(
                            out=up_w[:, a0:a0 + rc, py, 1:65, px],
                            in_=ps[:, :rc, :],
                        )
                if a0 == 0:
                    # zero the bottom two halo rows on the r=0 strip (g=-2,-1):
                    # the phase formula extended below the image is invalid there
                    nc.vector.tensor_scalar_mul(up_r[:, 0:2, :], up_r[:, 0:2, :],
                                                mk_t[:, 0:1])
                if fp8r:
                    for dx in need_dx:
                        nc.scalar.copy(
                            out=upf_t[:, dx, 2 * a0:2 * a0 + 12, :],
                            in_=up_r[:, 2 * a0:2 * a0 + 12, dx:dx + W])

            # ---- stage B: effective-cell conv -> out ----
            if fp8r:
                _stage_b_fp8r(nc, tc, up_r, upf_t, wb_t, wr_t, bigs, pairs,
                              psB, psR, outp, out_d)
            elif hyb:
                with (
                    tc.tile_pool(name="vyp", bufs=2) as vyp,
                    tc.tile_pool(name="smp", bufs=2) as smp,
                ):
                    # per-tap (row, col) bases into vector-engine registers
                    rvs = [nc.vector.value_load(co_t[0:1, k, 0:1],
                                                min_val=0, max_val=3)
                           for k in range(9)]
                    cvs = [nc.vector.value_load(co_t[0:1, k, 1:2],
                                                min_val=0, max_val=3)
                           for k in range(9)]
                    mm = mybir.AluOpType.mult
                    aa = mybir.AluOpType.add
                    up_fl = up_full[:]
                    for sb in range(OUT_R // SBR):
                        vys, samps = [], []
                        for k in range(9):
                            vy = vyp.tile([COUT, SBR, UC], mybir.dt.bfloat16,
                                          tag=f"vy{k}")
                            # [SBR rows x UC cols] shifted window == contiguous
                            # flat block of SBR*UC elements
                            base = rvs[k] * UC + cvs[k] + (SBR * sb) * UC
                            i0 = up_fl[:, bass.ds(base, SBR * UC)].rearrange(
                                "p (a b) -> p a b", b=UC)
                            i1 = up_fl[:, bass.ds(base + UC, SBR * UC)].rearrange(
                                "p (a b) -> p a b", b=UC)
                            nc.vector.tensor_scalar_mul(vy[:], i0, bs_t[:, k, 0:1])
                            nc.vector.scalar_tensor_tensor(
                                out=vy[:], in0=i1, scalar=bs_t[:, k, 1:2],
                                in1=vy[:], op0=mm, op1=aa)
                            vys.append(vy)
                        for k in range(S):
                            sa = smp.tile([COUT, SBR, W], mybir.dt.bfloat16,
                                          tag=f"sa{k}")
                            nc.vector.tensor_scalar_mul(
                                sa[:], vys[k][:, :, 0:W], dsc_t[:, k, 0:1])
                            nc.vector.scalar_tensor_tensor(
                                out=sa[:], in0=vys[k][:, :, 1:W + 1],
                                scalar=dsc_t[:, k, 1:2], in1=sa[:],
                                op0=mm, op1=aa)
                            samps.append(sa)
                        for sub in range(SBR // RBLK):
                            rs = slice(RBLK * sub, RBLK * (sub + 1))
                            bi = (SBR * sb) // RBLK + sub
                            for half in range(2):
                                ps = psB.tile([COUT, RBLK, W], mybir.dt.float32,
                                              tag="psB")
                                nmm = S + 2 * (9 - S)
                                si = 0
                                for k in range(9):
                                    if k < S:
                                        rhss = [samps[k][:, rs, :]]
                                    else:
                                        rhss = [vys[k][:, rs, 0:W],
                                                vys[k][:, rs, 1:W + 1]]
                                    for rhs in rhss:
                                        nc.tensor.matmul(
                                            ps[:], lhsT=wb_t[:, si, half, :],
                                            rhs=rhs, start=(si == 0),
                                            stop=(si == nmm - 1))
                                        si += 1
                                ob = outp.tile([COUT, RBLK, W], mybir.dt.float32,
                                               tag="ob")
                                nc.scalar.copy(out=ob[:], in_=ps[:])
                                nc.sync.dma_start(
                                    out=out_d[128 * half:128 * (half + 1),
                                              RBLK * bi:RBLK * (bi + 1), :],
                                    in_=ob[:])
            else:
                if dyn:
                    # per-slot (row, col) bases into tensor-engine registers
                    rvs = [nc.tensor.value_load(co_t[0:1, ci, 0:1],
                                                min_val=0, max_val=4)
                           for ci in range(ncell)]
                    cvs = [nc.tensor.value_load(co_t[0:1, ci, 1:2],
                                                min_val=0, max_val=4)
                           for ci in range(ncell)]
                for bi in range(OUT_R // RBLK):
                    for half in range(2):
                        ps = psB.tile([COUT, RBLK, W], mybir.dt.float32, tag="psB")
                        for ci in range(ncell):
                            if dyn:
                                rhs = up_r[:, bass.ds(rvs[ci] + 4 * bi, RBLK),
                                           bass.ds(cvs[ci], W)]
                            else:
                                dyi, dxi = cells[ci] // 5, cells[ci] % 5
                                ys = 4 * bi + dyi  # up row = o_l + 2 + (dyi-2)
                                rhs = up_r[:, ys:ys + RBLK, dxi:dxi + W]
                            nc.tensor.matmul(
                                ps[:],
                                lhsT=wb_t[:, ci, half, :],
                                rhs=rhs,
                                start=(ci == 0), stop=(ci == ncell - 1),
                            )
                        ob = outp.tile([COUT, RBLK, W], mybir.dt.float32, tag="ob")
                        nc.scalar.copy(out=ob[:], in_=ps[:])
                        nc.sync.dma_start(
                            out=out_d[128 * half:128 * (half + 1),
                                      4 * bi:4 * bi + RBLK, :],
                            in_=ob[:],
                        )

    nc.compile()
    return nc


def _stage_b_fp8r(nc, tc, up_r, upf_t, wb_t, wr_t, bigs, pairs,
                  psB, psR, outp, out_d):
    """Stage B with big cells in bf16 and ring-cell pairs in fp8 DoubleRow.

    upf_t[dx] holds a margin-free fp8 copy of up cols [dx, dx+128), so every
    cell window is a contiguous 512-element block and pair steps are
    automatically 16-aligned (multiples of 128)."""
    mm = mybir.AluOpType.mult
    aa = mybir.AluOpType.add

    # (fp8 casts of up are emitted inline with stage A, band by band)

    upf_fl = upf_t[:].rearrange("p a b c -> p (a b c)")

    def cell_off(c, bi):
        return (c % 5) * (UR * W) + ((4 * bi) + (c // 5)) * W

    G = 2  # blocks per weight-reuse group
    for half in range(2):
        for bg in range(OUT_R // RBLK // G):
            pscs = [psB.tile([COUT, RBLK, W], mybir.dt.float32, tag="psB",
                             name=f"psc_{half}_{bg}_{g}") for g in range(G)]
            for si, ci in enumerate(bigs):
                dyi, dxi = ci // 5, ci % 5
                for g in range(G):
                    bi = G * bg + g
                    ys = 4 * bi + dyi
                    nc.tensor.matmul(
                        pscs[g][:], lhsT=wb_t[:, si, half, :],
                        rhs=up_r[:, ys:ys + RBLK, dxi:dxi + W],
                        start=(si == 0), stop=(si == len(bigs) - 1))
            psrs = None
            if pairs:
                psrs = [psR.tile([COUT, RBLK, W], mybir.dt.float32, tag="psR",
                                 name=f"psr_{half}_{bg}_{g}") for g in range(G)]
                for p, (c1, c2) in enumerate(pairs):
                    step = cell_off(c2, 0) - cell_off(c1, 0)
                    assert step > 0 and step % 16 == 0
                    for g in range(G):
                        bi = G * bg + g
                        win = upf_fl[:, cell_off(c1, bi):cell_off(c1, bi) + RBLK * W]
                        rhs = bass.AP(tensor=win.tensor, offset=win.offset,
                                      ap=[win.ap[0], [step, 2], win.ap[1]])
                        nc.tensor.matmul(
                            psrs[g][:], lhsT=wr_t[:, p, half, :, :], rhs=rhs,
                            perf_mode=mybir.MatmulPerfMode.DoubleRow,
                            start=(p == 0), stop=(p == len(pairs) - 1))
            for g in range(G):
                bi = G * bg + g
                ob = outp.tile([COUT, RBLK, W], mybir.dt.float32, tag="ob")
                nc.scalar.copy(out=ob[:], in_=pscs[g][:])
                if pairs:
                    # TensorScalarPtr may read only one PSUM input
                    nc.vector.scalar_tensor_tensor(
                        out=ob[:], in0=psrs[g][:], scalar=1.0 / RING_SCALE,
                        in1=ob[:], op0=mm, op1=aa)
                nc.sync.dma_start(
                    out=out_d[128 * half:128 * (half + 1),
                              RBLK * bi:RBLK * (bi + 1), :],
                    in_=ob[:])


# --------------------------------------------------------------------------
# entry point
# --------------------------------------------------------------------------

def kernel(x, lateral_feat, trans_w, off_w1, off_b1, off_w2, off_b2):
    x = np.asarray(x)
    oy, ox = _offsets_from_inputs(np.asarray(lateral_feat), np.asarray(off_w1),
                                  np.asarray(off_b1), np.asarray(off_w2),
                                  np.asarray(off_b2))
    in_maps, ncell = _prep_in_maps(x, np.asarray(trans_w), oy, ox)

    key = (VARIANT, ncell)
    if key not in _CACHED_NC:
        _CACHED_NC[key] = _build_nc(ncell)
    nc = _CACHED_NC[key]

    res = run_bass_kernel_spmd(nc, in_maps, core_ids=list(range(N_CORES)))

    out = np.empty((N_BATCH, CIN, H, W), np.float32)
    for core in range(N_CORES):
        n, r = core // STRIPS, core % STRIPS
        out[n, :, OUT_R * r:OUT_R * (r + 1), :] = res.results[core]["out"]
    return out



# revision 17
# speedup vs baseline: 1.1959x; 1.1959x over previous
"""Trainium2 Bass kernel for nn_DeformableTransposedConv.

Pipeline (per the reference):
  up  = ConvTranspose2d(x, trans_w, stride=2, pad=1, outpad=1)   # [N,128,128,128]
  off = tanh(conv(relu(conv(lateral_feat, w1)), w2))             # [N,18,1,1] -> broadcast
  out = deform_conv2d(up, off, trans_w, pad=1)                   # [N,256,128,128]

Key structure exploited:
  * The offsets are constant over space (1x1 lateral input broadcast), so the
    bilinear deformable gather collapses to a per-batch 5x5 conv with
    "effective" weights W_eff[n] built host-side from trans_w and the (tiny)
    offsets.  The device computes:
        out[n] = sum_{dy,dx in 5x5} W_eff[n,dy,dx] @ shift(up[n], dy, dx)
    as PSUM-accumulated matmuls over the 128 up-channels.
  * The stride-2 transposed conv splits into 4 phase sub-convs with
    {1,2,2,4} taps, each a PSUM-accumulated matmul over the 256 x-channels.

Sharding: 8 cores = 2 batches x 4 row-strips of 32 output rows.  Each core
computes out[n, :, 32r:32r+32, :] from a 20-row slice of x (with halo).
All weights / layout prep / zero padding is done host-side; the NEFF is
input-independent (weights and data are ExternalInputs).
"""

import numpy as np
import ml_dtypes

import concourse.bass as bass
import concourse.tile as tile
from concourse import bacc, mybir
from concourse.bass_utils import run_bass_kernel_spmd

BF16 = ml_dtypes.bfloat16

# ---- problem constants (hardcoded per contract) ----
N_BATCH = 2
CIN = 256
COUT = 128          # up channels
K = 3
PAD = 1
H0 = W0 = 64        # x spatial
H = W = 128         # up / out spatial
N_CORES = 8
STRIPS = 4          # row strips per batch
OUT_R = 32          # output rows per strip

# SBUF layout constants
XR, XC = 20, 66     # x tile rows (16 + 2 halo each side), cols (64 + 1 pad + 1 align)
UR, UC = 36, 132    # up tile rows (32 + 2 halo each side), cols (128 + 2 + 2)
NCELL = 25          # 5x5 effective deform kernel
RBLK = 4            # output rows per stage-B block (4*128 = 512 = one PSUM bank)

# stage-B variant:
#   "full25" = static 5x5 effective conv (25 matmul terms / block)
#   "slots"  = dynamic cell slots (pruned zero cells, runtime offsets)
#   "hybrid" = DVE bilinear blends + matmuls (y-blend on DVE for all taps;
#              x-blend on DVE for the first HYBRID_S taps, folded into scaled
#              weights for the rest)
import os as _os
VARIANT = _os.environ.get("KERNEL_VARIANT", "v2")
HYBRID_S = int(_os.environ.get("HYBRID_S", "4"))
SBR = 8             # hybrid blend superblock rows (2 PSUM blocks)
PLR = 32            # v2: ring-plane rows (blocks 0..7 read plane rows 4bi..4bi+3)
V2_VEC_PLANES = int(_os.environ.get("V2_VEC_PLANES", "1"))  # 0=scalar,1=split,2=vector
V2_MIXED_GROUP = _os.environ.get("V2_MIXED_GROUP", "1") == "1"
V2_OUT = _os.environ.get("V2_OUT", "f16")

_CACHED_NC = {}


# --------------------------------------------------------------------------
# host-side preparation
# --------------------------------------------------------------------------

def _offsets_from_inputs(lateral_feat, off_w1, off_b1, off_w2, off_b2):
    """Tiny offset MLP (conv on 1x1 spatial input == center-tap matmul)."""
    lf = lateral_feat[:, :, 0, 0].astype(np.float32)                    # [N,128]
    h = np.maximum(0.0, lf @ off_w1[:, :, 1, 1].T.astype(np.float32)
                   + off_b1.astype(np.float32))                         # [N,64]
    off = np.tanh(h @ off_w2[:, :, 1, 1].T.astype(np.float32)
                  + off_b2.astype(np.float32)).astype(np.float32)       # [N,18]
    oy = off.reshape(-1, K * K, 2)[:, :, 0]
    ox = off.reshape(-1, K * K, 2)[:, :, 1]
    return oy, ox


def _w_eff(trans_w, oy, ox):
    """Effective 5x5 deform weights. Returns [N, 5, 5, 256(o), 128(c)] f32."""
    n_b = oy.shape[0]
    Weff = np.zeros((n_b, 5, 5, CIN, COUT), np.float32)
    for n in range(n_b):
        for k in range(K * K):
            ky, kx = k // K, k % K
            ay = np.float32(ky - 1) + oy[n, k]
            ax = np.float32(kx - 1) + ox[n, k]
            Ay, Ax = int(np.floor(ay)), int(np.floor(ax))
            dy = float(ay) - Ay
            dx = float(ax) - Ax
            tap = trans_w[:, :, ky, kx].astype(np.float32)
            for cy, wy in ((0, 1.0 - dy), (1, dy)):
                for cx, wx in ((0, 1.0 - dx), (1, dx)):
                    w = wy * wx
                    if w != 0.0:
                        Weff[n, Ay + cy + 2, Ax + cx + 2] += w * tap
    return Weff


def _prep_in_maps(x, trans_w, oy, ox):
    """Build the per-core input dicts (already bf16, padded, SBUF-layouts).
    Returns (in_maps, ncell) where ncell is the stage-B slot count."""
    xf = x.astype(np.float32)

    # stage-A weights, shared by all cores: wa[k, h2, j, m]
    wa = np.zeros((COUT, 2, 9, COUT), np.float32)
    for h2 in range(2):
        for j in range(9):
            jy, jx = j // 3, j % 3
            # lhsT[K=cin(128), M=cout(128)] = trans_w[h2*128+kk, m, jy, jx]
            wa[:, h2, j, :] = trans_w[h2 * 128:(h2 + 1) * 128, :, jy, jx]
    wa_b = wa.astype(BF16).reshape(COUT, 2 * 9 * COUT)

    if VARIANT == "hybrid":
        return _prep_in_maps_hybrid(xf, trans_w, oy, ox, wa_b)
    if VARIANT == "fp8r":
        return _prep_in_maps_fp8r(xf, trans_w, oy, ox, wa_b)
    if VARIANT == "v2":
        return _prep_in_maps_v2(xf, trans_w, oy, ox, wa_b)

    # stage-B weights per batch
    Weff = _w_eff(trans_w, oy, ox)                      # [N,5,5,256,128]
    wb_all, co_all = [], []
    if VARIANT == "full25":
        ncell = NCELL
        for n in range(N_BATCH):
            wb = Weff[n].reshape(NCELL, 2, COUT, COUT)   # [cell, half, o(128), c]
            wb = wb.transpose(3, 0, 1, 2)                # [c, cell, half, o]
            wb_all.append(np.ascontiguousarray(wb).astype(BF16)
                          .reshape(COUT, NCELL * 2 * COUT))
            co_all.append(None)
    elif VARIANT == "union":
        # static program specialized on the union of nonzero cells across
        # batches (compile cache keyed on the union tuple)
        nz = [np.nonzero(np.abs(Weff[n]).reshape(25, -1).max(1) > 0)[0]
              for n in range(N_BATCH)]
        union = sorted(set(int(c) for z in nz for c in z))
        ncell = ("union",) + tuple(union)
        for n in range(N_BATCH):
            wb = np.zeros((len(union), 2, COUT, COUT), np.float32)
            for s, ci in enumerate(union):
                wb[s] = Weff[n, ci // 5, ci % 5].reshape(2, COUT, COUT)
            wb = wb.transpose(3, 0, 1, 2)
            wb_all.append(np.ascontiguousarray(wb).astype(BF16)
                          .reshape(COUT, len(union) * 2 * COUT))
            co_all.append(None)
    else:  # "slots": pruned nonzero cells, offsets shipped as data
        nz = [np.nonzero(np.abs(Weff[n]).reshape(25, -1).max(1) > 0)[0]
              for n in range(N_BATCH)]
        ncell = max(len(z) for z in nz)
        for n in range(N_BATCH):
            cells = list(nz[n]) + [12] * (ncell - len(nz[n]))  # pad w/ center
            wb = np.zeros((ncell, 2, COUT, COUT), np.float32)
            co = np.zeros((1, ncell, 2), np.int32)
            for s, ci in enumerate(cells):
                dyi, dxi = ci // 5, ci % 5
                if s < len(nz[n]):
                    wb[s] = Weff[n, dyi, dxi].reshape(2, COUT, COUT)
                co[0, s] = (dyi, dxi)
            wb = wb.transpose(3, 0, 1, 2)                # [c, slot, half, o]
            wb_all.append(np.ascontiguousarray(wb).astype(BF16)
                          .reshape(COUT, ncell * 2 * COUT))
            co_all.append(co)

    in_maps = []
    for core in range(N_CORES):
        n, r = core // STRIPS, core % STRIPS
        # x slice with halo: global x rows [16r-2, 16r+18)
        xs = np.zeros((COUT, 2, XR, XC), np.float32)
        r0 = 16 * r - 2
        lo, hi = max(0, r0), min(H0, r0 + XR)
        for h2 in range(2):
            xs[:, h2, lo - r0:hi - r0, :W0] = xf[n, h2 * 128:(h2 + 1) * 128, lo:hi, :]
        # bottom-halo validity mask: strip 0 must zero up rows g=-2,-1 which
        # the phase formula would otherwise fill with spurious values
        mk = np.full((COUT, 1), 0.0 if r == 0 else 1.0, np.float32)
        im = {
            "xs": np.ascontiguousarray(xs.astype(BF16).reshape(COUT, 2 * XR * XC)),
            "wa": wa_b,
            "wb": wb_all[n],
            "mk": mk,
        }
        if co_all[n] is not None:
            im["co"] = co_all[n]
        in_maps.append(im)
    return in_maps, ncell


FP8 = ml_dtypes.float8_e4m3
RING_SCALE = 256.0


def _prep_in_maps_fp8r(xf, trans_w, oy, ox, wa_b):
    """Union cells; big cells in bf16, small 'ring' cells paired into fp8
    DoubleRow matmuls (weights scaled by RING_SCALE)."""
    Weff = _w_eff(trans_w, oy, ox)                       # [N,5,5,256,128]
    norms = np.abs(Weff).reshape(N_BATCH, 25, -1).max(2)  # [N,25]
    union = sorted(set(np.nonzero(norms.max(0) > 0)[0].tolist()))
    thr = 0.25 * norms.max()
    bigs = [c for c in union if norms[:, c].max() > thr]
    rings = [c for c in union if c not in bigs]
    if len(rings) % 2:
        bigs.append(rings.pop())                          # odd leftover -> bf16
    # order by window offset (dx major, dy minor); pair far-apart cells so the
    # two DoubleRow K-group windows never overlap (overlapping windows were
    # measured ~1.7x slower on the PE)
    rings.sort(key=lambda c: (c % 5, c // 5))
    nh = len(rings) // 2
    pairs = [(rings[i], rings[i + nh]) for i in range(nh)]
    bigs = sorted(bigs)

    wb_all, wr_all = [], []
    for n in range(N_BATCH):
        wb = np.zeros((max(len(bigs), 1), 2, COUT, COUT), np.float32)
        for s, ci in enumerate(bigs):
            wb[s] = Weff[n, ci // 5, ci % 5].reshape(2, COUT, COUT)
        wb = wb.transpose(3, 0, 1, 2)                     # [c, slot, half, o]
        wb_all.append(np.ascontiguousarray(wb).astype(BF16)
                      .reshape(COUT, -1))
        wr = np.zeros((max(len(pairs), 1), 2, 2, COUT, COUT), np.float32)
        for p, (c1, c2) in enumerate(pairs):
            for half in range(2):
                wr[p, half, 0] = RING_SCALE * \
                    Weff[n, c1 // 5, c1 % 5][128 * half:128 * (half + 1)].T
                wr[p, half, 1] = RING_SCALE * \
                    Weff[n, c2 // 5, c2 % 5][128 * half:128 * (half + 1)].T
        # wr[p, half, ksub, c, o] -> [c, p, half, ksub, o]
        wr = wr.transpose(3, 0, 1, 2, 4)
        wr_all.append(np.ascontiguousarray(wr).astype(FP8).reshape(COUT, -1))

    in_maps = []
    for core in range(N_CORES):
        n, r = core // STRIPS, core % STRIPS
        xs = np.zeros((COUT, 2, XR, XC), np.float32)
        r0 = 16 * r - 2
        lo, hi = max(0, r0), min(H0, r0 + XR)
        for h2 in range(2):
            xs[:, h2, lo - r0:hi - r0, :W0] = xf[n, h2 * 128:(h2 + 1) * 128, lo:hi, :]
        mk = np.full((COUT, 1), 0.0 if r == 0 else 1.0, np.float32)
        in_maps.append({
            "xs": np.ascontiguousarray(xs.astype(BF16).reshape(COUT, 2 * XR * XC)),
            "wa": wa_b,
            "wb": wb_all[n],
            "wr": wr_all[n],
            "mk": mk,
        })
    return in_maps, ("fp8r", tuple(bigs), tuple(pairs))


RING_W_SCALE = 16.0     # ring weights x16, up fp8 copies x1/16 -> product x1


def _prep_in_maps_v2(xf, trans_w, oy, ox, wa_b):
    """v3: static big cells + static union ring cells, both accumulated into
    ONE psum bank per output block.

    Ring cells (bilinear spill corners) are paired into fp8 DoubleRow
    matmuls over per-dx margin-free fp8 copies of up.  Ring weights are
    scaled x16 and the fp8 copies x1/16, so the pair product is unscaled
    and rings accumulate into the SAME psum bank as the big cells (no
    separate merge pass).  Cells whose max-norm is below 0.4% of the
    global max (the ab bilinear corners, ~1e-4 relative) are dropped
    (~0.1% output error)."""
    Weff = _w_eff(trans_w, oy, ox)                        # [N,5,5,256,128]
    norms = np.abs(Weff).reshape(N_BATCH, 25, -1).max(2)  # [N,25]
    gmax = norms.max()
    bigs = sorted(int(c) for c in np.nonzero(norms.max(0) > 0.25 * gmax)[0])
    keep = (norms.max(0) > 0.004 * gmax) & (norms.max(0) <= 0.25 * gmax)
    rings = [int(c) for c in np.nonzero(keep)[0] if c not in bigs]
    # order by (dx major, dy minor) and pair far apart so the two DoubleRow
    # K-group windows never overlap
    rings.sort(key=lambda c: (c % 5, c // 5))
    if len(rings) % 2:
        # pad slot: any distinct cell position (zero weights, contributes 0)
        pad = next(c for c in range(25) if c not in rings)
        rings.append(pad)
        rings.sort(key=lambda c: (c % 5, c // 5))
    nh = len(rings) // 2
    pairs = [(rings[i], rings[i + nh]) for i in range(nh)]
    need_dx = sorted({c % 5 for c in rings})
    dx_slot = {d: i for i, d in enumerate(need_dx)}
    nbig = len(bigs)

    def cell_off(c, bi):
        return dx_slot[c % 5] * (UR * W) + (4 * bi + c // 5) * W

    # validate pair steps (static, positive, 16-aligned)
    for c1, c2 in pairs:
        step = cell_off(c2, 0) - cell_off(c1, 0)
        assert step > 0 and step % 16 == 0, (c1, c2, step)

    wb_all, wr_all = [], []
    for n in range(N_BATCH):
        wb = np.zeros((2, nbig, COUT, COUT), np.float32)  # [half, s, o, c]
        for s, ci in enumerate(bigs):
            wb[:, s] = Weff[n, ci // 5, ci % 5].reshape(2, COUT, COUT)
        wb = wb.transpose(3, 0, 1, 2)                     # [c, half, s, o]
        wb_all.append(np.ascontiguousarray(wb).astype(BF16).reshape(COUT, -1))
        wr = np.zeros((max(len(pairs), 1), 2, 2, COUT, COUT), np.float32)
        for p, (c1, c2) in enumerate(pairs):
            for half in range(2):
                wr[p, half, 0] = RING_W_SCALE * \
                    Weff[n, c1 // 5, c1 % 5][128 * half:128 * (half + 1)].T
                wr[p, half, 1] = RING_W_SCALE * \
                    Weff[n, c2 // 5, c2 % 5][128 * half:128 * (half + 1)].T
        wr = wr.transpose(3, 0, 1, 2, 4)                  # [c, p, half, ksub, o]
        wr_all.append(np.ascontiguousarray(wr).astype(FP8).reshape(COUT, -1))

    in_maps = []
    for core in range(N_CORES):
        n, r = core // STRIPS, core % STRIPS
        xs = np.zeros((COUT, 2, XR, XC), np.float32)
        r0 = 16 * r - 2
        lo, hi = max(0, r0), min(H0, r0 + XR)
        for h2 in range(2):
            xs[:, h2, lo - r0:hi - r0, :W0] = xf[n, h2 * 128:(h2 + 1) * 128, lo:hi, :]
        mk = np.full((COUT, 1), 0.0 if r == 0 else 1.0, np.float32)
        in_maps.append({
            "xs": np.ascontiguousarray(xs.astype(BF16).reshape(COUT, 2 * XR * XC)),
            "wa": wa_b,
            "wb": wb_all[n],
            "wr": wr_all[n],
            "mk": mk,
        })
    return in_maps, ("v2", tuple(bigs), tuple(pairs), tuple(need_dx))


def _prep_in_maps_hybrid(xf, trans_w, oy, ox, wa_b):
    S = HYBRID_S
    nslot = S + 2 * (9 - S)
    wb_all, bs_all, dsc_all, ofs_all = [], [], [], []
    for n in range(N_BATCH):
        wb = np.zeros((nslot, 2, COUT, COUT), np.float32)   # [slot, half, c, o]
        bs = np.zeros((9, 2), np.float32)
        dsc = np.zeros((max(S, 1), 2), np.float32)
        ofs = np.zeros((1, 9, 2), np.int32)
        for k in range(9):
            ky, kx = k // 3, k % 3
            ay = np.float32(ky - 1) + oy[n, k]
            ax = np.float32(kx - 1) + ox[n, k]
            Ay, Ax = int(np.floor(ay)), int(np.floor(ax))
            dy = float(ay) - Ay
            dx = float(ax) - Ax
            ofs[0, k] = (2 + Ay, 2 + Ax)
            bs[k] = (1.0 - dy, dy)
            wkT = np.stack([trans_w[h * 128:(h + 1) * 128, :, ky, kx].T
                            for h in range(2)])             # [half, c, o]
            if k < S:
                dsc[k] = (1.0 - dx, dx)
                wb[k] = wkT
            else:
                wb[S + 2 * (k - S) + 0] = (1.0 - dx) * wkT
                wb[S + 2 * (k - S) + 1] = dx * wkT
        wb = wb.transpose(2, 0, 1, 3)                       # [c, slot, half, o]
        wb_all.append(np.ascontiguousarray(wb).astype(BF16)
                      .reshape(COUT, nslot * 2 * COUT))
        bs_all.append(np.broadcast_to(bs.reshape(1, 9, 2),
                                      (COUT, 9, 2)).copy())
        dsc_all.append(np.broadcast_to(dsc.reshape(1, -1, 2),
                                       (COUT, max(S, 1), 2)).copy())
        ofs_all.append(ofs)

    in_maps = []
    for core in range(N_CORES):
        n, r = core // STRIPS, core % STRIPS
        xs = np.zeros((COUT, 2, XR, XC), np.float32)
        r0 = 16 * r - 2
        lo, hi = max(0, r0), min(H0, r0 + XR)
        for h2 in range(2):
            xs[:, h2, lo - r0:hi - r0, :W0] = xf[n, h2 * 128:(h2 + 1) * 128, lo:hi, :]
        mk = np.full((COUT, 1), 0.0 if r == 0 else 1.0, np.float32)
        in_maps.append({
            "xs": np.ascontiguousarray(xs.astype(BF16).reshape(COUT, 2 * XR * XC)),
            "wa": wa_b,
            "wb": wb_all[n],
            "mk": mk,
            "bs": bs_all[n].reshape(COUT, 18),
            "dsc": dsc_all[n].reshape(COUT, -1),
            "co": ofs_all[n],
        })
    return in_maps, nslot


# --------------------------------------------------------------------------
# device program (input-independent; same for all cores except r-dependent
# row validity -> handled by *uniform* structure: we compute all 36 up rows,
# rows outside [0,128) stay zero because their x inputs are zeroed host-side
# ... except parity bookkeeping differs per strip; we keep the program truly
# SPMD by computing the full 18 a'-rows per phase and masking via zero x.)
# --------------------------------------------------------------------------

def _build_nc_v2(key):
    """v3 device program: interleaved stage A bands / stage B block groups,
    static big + ring cells unified into one psum bank, fp16 output."""
    _, bigs, pairs, need_dx = key
    bigs, pairs, need_dx = list(bigs), list(pairs), list(need_dx)
    nbig, npair, ndx = len(bigs), len(pairs), len(need_dx)
    dx_slot = {d: i for i, d in enumerate(need_dx)}
    nc = bacc.Bacc("TRN2", target_bir_lowering=False, debug=False,
                   enable_asserts=False)

    xs_d = nc.dram_tensor("xs", [COUT, 2 * XR * XC], mybir.dt.bfloat16,
                          kind="ExternalInput").ap()
    wa_d = nc.dram_tensor("wa", [COUT, 2 * 9 * COUT], mybir.dt.bfloat16,
                          kind="ExternalInput").ap()
    wb_d = nc.dram_tensor("wb", [COUT, 2 * nbig * COUT], mybir.dt.bfloat16,
                          kind="ExternalInput").ap()
    wr_d = nc.dram_tensor("wr", [COUT, max(npair, 1) * 2 * 2 * COUT],
                          mybir.dt.float8e4, kind="ExternalInput").ap()
    mk_d = nc.dram_tensor("mk", [COUT, 1], mybir.dt.float32,
                          kind="ExternalInput").ap()
    out_dt = {"f16": mybir.dt.float16, "bf16": mybir.dt.bfloat16,
              "f32": mybir.dt.float32}[V2_OUT]
    out_d = nc.dram_tensor("out", [CIN, OUT_R, W], out_dt,
                           kind="ExternalOutput").ap()

    with tile.TileContext(nc) as tc:
        with (
            tc.tile_pool(name="singles", bufs=1) as singles,
            tc.tile_pool(name="outp", bufs=4) as outp,
            tc.tile_pool(name="psA", bufs=4, space="PSUM") as psA,
            tc.tile_pool(name="psB", bufs=4, space="PSUM") as psB,
        ):
            xs_t = singles.tile([COUT, 2, XR, XC], mybir.dt.bfloat16)
            wa_t = singles.tile([COUT, 2, 9, COUT], mybir.dt.bfloat16)
            wb_t = singles.tile([COUT, 2, nbig, COUT], mybir.dt.bfloat16)
            wr_t = singles.tile([COUT, max(npair, 1), 2, 2, COUT],
                                mybir.dt.float8e4)
            mk_t = singles.tile([COUT, 1], mybir.dt.float32)
            up_full = singles.tile([COUT, UR * UC + 12], mybir.dt.bfloat16)
            up_t = up_full[:, :UR * UC]
            upf_t = singles.tile([COUT, max(ndx, 1), UR, W], mybir.dt.float8e4)

            # ---- input DMA, ordered so the first stage-A matmuls (half 0,
            # band 0) can start as early as possible ----
            xs4 = xs_t[:]
            xs4_d = xs_d.rearrange("p (a b c) -> p a b c", a=2, b=XR)
            wa_flat = wa_t[:].rearrange("p a b c -> p (a b c)")
            wb_flat = wb_t[:].rearrange("p a b c -> p (a b c)")
            # scalar HWDGE queue: xs band0 halves, wb half0, xs rest, wb h1
            nc.scalar.dma_start(out=xs4[:, 0, 0:9, :], in_=xs4_d[:, 0, 0:9, :])
            nc.scalar.dma_start(out=xs4[:, 1, 0:9, :], in_=xs4_d[:, 1, 0:9, :])
            nc.scalar.dma_start(out=wb_flat[:, :nbig * COUT],
                                in_=wb_d[:, :nbig * COUT])
            nc.scalar.dma_start(out=xs4[:, 0, 9:, :], in_=xs4_d[:, 0, 9:, :])
            nc.scalar.dma_start(out=xs4[:, 1, 9:, :], in_=xs4_d[:, 1, 9:, :])
            nc.scalar.dma_start(out=wb_flat[:, nbig * COUT:],
                                in_=wb_d[:, nbig * COUT:])
            # sync HWDGE queue: wa halves, wr, mk
            nc.sync.dma_start(out=wa_flat[:, :9 * COUT], in_=wa_d[:, :9 * COUT])
            nc.sync.dma_start(out=wa_flat[:, 9 * COUT:], in_=wa_d[:, 9 * COUT:])
            nc.sync.dma_start(
                out=wr_t[:].rearrange("p a b c d -> p (a b c d)"), in_=wr_d)
            nc.sync.dma_start(out=mk_t[:], in_=mk_d)

            nc.vector.memset(up_full[:], 0.0)

            up_w = up_t.rearrange("p (a q c r) -> p a q c r", q=2, c=66, r=2)
            up_r = up_t.rearrange("p (l c) -> p l c", c=132)
            upf_fl = upf_t[:].rearrange("p a b c -> p (a b c)")

            ytaps = {0: ((1, 0),), 1: ((2, 0), (0, 1))}
            band_blocks = ((0, 1), (2, 3, 4), (5, 6, 7))

            def cell_off(c, bi):
                return dx_slot[c % 5] * (UR * W) + (4 * bi + c // 5) * W

            for b in range(3):
                # ---- stage A band: up rows 12b .. 12b+11 ----
                a0 = 6 * b
                rc = 6
                for py in (0, 1):
                    for px in (0, 1):
                        taps = [(jy, dy, jx, dx)
                                for jy, dy in ytaps[py] for jx, dx in ytaps[px]]
                        ps = psA.tile([COUT, rc, 64], mybir.dt.float32,
                                      tag="psA")
                        nmm = len(taps) * 2
                        i = 0
                        for h2 in range(2):
                            for (jy, dy, jx, dx) in taps:
                                nc.tensor.matmul(
                                    ps[:, :rc, :],
                                    lhsT=wa_t[:, h2, jy * 3 + jx, :],
                                    rhs=xs_t[:, h2, a0 + 1 + dy:a0 + 1 + dy + rc,
                                             dx:dx + 64],
                                    start=(i == 0), stop=(i == nmm - 1),
                                )
                                i += 1
                        nc.scalar.copy(
                            out=up_w[:, a0:a0 + rc, py, 1:65, px],
                            in_=ps[:, :rc, :],
                        )
                if b == 0:
                    # zero the two bottom halo rows on the r=0 strip
                    nc.vector.tensor_scalar_mul(up_r[:, 0:2, :], up_r[:, 0:2, :],
                                                mk_t[:, 0:1])
                # ---- fp8 ring planes for this band (x 1/RING_W_SCALE) ----
                for i, dxp in enumerate(need_dx):
                    src = up_r[:, 12 * b:12 * b + 12, dxp:dxp + W]
                    dst = upf_t[:, i, 12 * b:12 * b + 12, :]
                    if V2_VEC_PLANES and i % 2 == 1:
                        nc.vector.tensor_scalar_mul(dst, src,
                                                    1.0 / RING_W_SCALE)
                    else:
                        nc.scalar.mul(out=dst, in_=src, mul=1.0 / RING_W_SCALE)
                # ---- stage B blocks now computable ----
                for bi in band_blocks[b]:
                    for half in range(2):
                        ps = psB.tile([COUT, RBLK, W], mybir.dt.float32,
                                      tag="psB")
                        nmm = nbig + npair
                        for s, ci in enumerate(bigs):
                            dyi, dxi = ci // 5, ci % 5
                            ys = 4 * bi + dyi
                            nc.tensor.matmul(
                                ps[:], lhsT=wb_t[:, half, s, :],
                                rhs=up_r[:, ys:ys + RBLK, dxi:dxi + W],
                                start=(s == 0), stop=(s == nmm - 1))
                        for p, (c1, c2) in enumerate(pairs):
                            step = cell_off(c2, 0) - cell_off(c1, 0)
                            off = cell_off(c1, bi)
                            win = upf_fl[:, off:off + RBLK * W]
                            rhs = bass.AP(tensor=win.tensor, offset=win.offset,
                                          ap=[win.ap[0], [step, 2], win.ap[1]])
                            nc.tensor.matmul(
                                ps[:], lhsT=wr_t[:, p, half, :, :], rhs=rhs,
                                perf_mode=mybir.MatmulPerfMode.DoubleRow,
                                start=False, stop=(nbig + p == nmm - 1))
                        ob = outp.tile([COUT, RBLK, W], out_dt, tag="ob")
                        nc.scalar.copy(out=ob[:], in_=ps[:])
                        nc.sync.dma_start(
                            out=out_d[128 * half:128 * (half + 1),
                                      RBLK * bi:RBLK * (bi + 1), :],
                            in_=ob[:])

    nc.compile()
    return nc


def _build_nc(ncell):
    if isinstance(ncell, tuple) and ncell[0] == "v2":
        return _build_nc_v2(ncell)
    fp8r = isinstance(ncell, tuple) and ncell[0] == "fp8r"
    if fp8r:
        bigs, pairs = list(ncell[1]), list(ncell[2])
        ncell = max(len(bigs), 1)
        cells, dyn = None, False
    elif isinstance(ncell, tuple):      # ("union", cell, cell, ...)
        cells = list(ncell[1:])
        ncell = len(cells)
        dyn = False
    else:
        cells = list(range(NCELL)) if VARIANT == "full25" else None
        dyn = VARIANT not in ("full25",)
    nc = bacc.Bacc("TRN2", target_bir_lowering=False, debug=False,
                   enable_asserts=False)

    xs_d = nc.dram_tensor("xs", [COUT, 2 * XR * XC], mybir.dt.bfloat16,
                          kind="ExternalInput").ap()
    wa_d = nc.dram_tensor("wa", [COUT, 2 * 9 * COUT], mybir.dt.bfloat16,
                          kind="ExternalInput").ap()
    wb_d = nc.dram_tensor("wb", [COUT, ncell * 2 * COUT], mybir.dt.bfloat16,
                          kind="ExternalInput").ap()
    mk_d = nc.dram_tensor("mk", [COUT, 1], mybir.dt.float32,
                          kind="ExternalInput").ap()
    if fp8r:
        wr_d = nc.dram_tensor(
            "wr", [COUT, max(len(pairs), 1) * 2 * 2 * COUT],
            mybir.dt.float8e4, kind="ExternalInput").ap()
    hyb = VARIANT == "hybrid"
    S = HYBRID_S
    if hyb:
        co_d = nc.dram_tensor("co", [1, 9, 2], mybir.dt.int32,
                              kind="ExternalInput").ap()
        bs_d = nc.dram_tensor("bs", [COUT, 18], mybir.dt.float32,
                              kind="ExternalInput").ap()
        dsc_d = nc.dram_tensor("dsc", [COUT, 2 * max(S, 1)], mybir.dt.float32,
                               kind="ExternalInput").ap()
    elif dyn:
        co_d = nc.dram_tensor("co", [1, ncell, 2], mybir.dt.int32,
                              kind="ExternalInput").ap()
    out_d = nc.dram_tensor("out", [CIN, OUT_R, W], mybir.dt.float32,
                           kind="ExternalOutput").ap()

    with tile.TileContext(nc) as tc:
        with (
            tc.tile_pool(name="singles", bufs=1) as singles,
            tc.tile_pool(name="outp", bufs=4) as outp,
            tc.tile_pool(name="psB", bufs=4, space="PSUM") as psB,
            tc.tile_pool(name="psR", bufs=4, space="PSUM") as psR,
        ):
            xs_t = singles.tile([COUT, 2, XR, XC], mybir.dt.bfloat16)
            wa_t = singles.tile([COUT, 2, 9, COUT], mybir.dt.bfloat16)
            wb_t = singles.tile([COUT, ncell, 2, COUT], mybir.dt.bfloat16)
            mk_t = singles.tile([COUT, 1], mybir.dt.float32)
            # +12 pad: hybrid vy reads may run a few elements past the last
            # row (col-window spill); padded region is zeroed, never consumed
            up_full = singles.tile([COUT, UR * UC + 12], mybir.dt.bfloat16)
            up_t = up_full[:, :UR * UC]

            # stage-A critical inputs split across both HWDGE queues; xs is
            # further split by row band so the first stage-A band can start
            # after ~0.3MB instead of the whole tensor.  Band a0 reads xs rows
            # a0+1+dy (dy<=1), so rows [0,9) cover band 0, [9,20) the rest.
            xs4 = xs_t[:]
            xs4_d = xs_d.rearrange("p (a b c) -> p a b c", a=2, b=XR)
            for h2 in range(2):
                eng = nc.sync if h2 == 0 else nc.scalar
                eng.dma_start(out=xs4[:, h2, 0:9, :], in_=xs4_d[:, h2, 0:9, :])
            nc.sync.dma_start(out=wa_t[:].rearrange("p a b c -> p (a b c)"), in_=wa_d)
            for h2 in range(2):
                eng = nc.scalar if h2 == 0 else nc.sync
                eng.dma_start(out=xs4[:, h2, 9:, :], in_=xs4_d[:, h2, 9:, :])
            nc.sync.dma_start(out=mk_t[:], in_=mk_d)
            wb_flat = wb_t[:].rearrange("p a b c -> p (a b c)")
            nc.scalar.dma_start(out=wb_flat, in_=wb_d)
            if fp8r:
                wr_t = singles.tile([COUT, max(len(pairs), 1), 2, 2, COUT],
                                    mybir.dt.float8e4)
                nc.sync.dma_start(
                    out=wr_t[:].rearrange("p a b c d -> p (a b c d)"), in_=wr_d)
                upf_t = singles.tile([COUT, 5, UR, W], mybir.dt.float8e4)
            if hyb:
                co_t = singles.tile([1, 9, 2], mybir.dt.int32)
                bs_t = singles.tile([COUT, 9, 2], mybir.dt.float32)
                dsc_t = singles.tile([COUT, max(S, 1), 2], mybir.dt.float32)
                nc.sync.dma_start(out=co_t[:].rearrange("p a b -> p (a b)"),
                                  in_=co_d.rearrange("p a b -> p (a b)"))
                nc.sync.dma_start(out=bs_t[:].rearrange("p a b -> p (a b)"),
                                  in_=bs_d)
                nc.sync.dma_start(out=dsc_t[:].rearrange("p a b -> p (a b)"),
                                  in_=dsc_d)
            elif dyn:
                co_t = singles.tile([1, ncell, 2], mybir.dt.int32)
                nc.sync.dma_start(out=co_t[:].rearrange("p a b -> p (a b)"),
                                  in_=co_d.rearrange("p a b -> p (a b)"))

            # zero the up tile (margins + potentially-invalid rows)
            nc.vector.memset(up_full[:], 0.0)

            # views of up: [p, a'(18), q(2), cc(66), r(2)] for phase writes,
            # [p, l(36), c(132)] for stage-B reads
            up_w = up_t.rearrange("p (a q c r) -> p a q c r", q=2, c=66, r=2)
            up_r = up_t.rearrange("p (l c) -> p l c", c=132)

            # ---- stage A: transposed conv -> up ----
            # row-major (a0 outer) so each 12-row band of up completes early;
            # for fp8r the band's fp8 casts are emitted right behind it, so
            # the ring matmuls never wait on a late cast burst
            ytaps = {0: ((1, 0),), 1: ((2, 0), (0, 1))}
            if fp8r:
                need_dx = sorted({c % 5 for pr in pairs for c in pr})
            for a0 in range(0, 18, 6):
                rc = 6
                for py in (0, 1):
                    for px in (0, 1):
                        taps = [(jy, dy, jx, dx)
                                for jy, dy in ytaps[py] for jx, dx in ytaps[px]]
                        # stage A borrows the ring pool (idle here) so its
                        # evacuations never block stage-B big-cell psum slots
                        pool = psR if fp8r else psB
                        ps = pool.tile([COUT, 6, 64], mybir.dt.float32,
                                       tag="psR" if fp8r else "psB")
                        nmm = len(taps) * 2
                        i = 0
                        for (jy, dy, jx, dx) in taps:
                            for h2 in range(2):
                                nc.tensor.matmul(
                                    ps[:, :rc, :],
                                    lhsT=wa_t[:, h2, jy * 3 + jx, :],
                                    rhs=xs_t[:, h2, a0 + 1 + dy:a0 + 1 + dy + rc,
                                             dx:dx + 64],
                                    start=(i == 0), stop=(i == nmm - 1),
                                )
                                i += 1
                        # scatter phase result into up (cast to bf16)
                        nc.scalar.copy(
                            out=up_w[:, a0:a0 + rc, py, 1:65, px],
                            in_=ps[:, :rc, :],
                        )
                if a0 == 0:
                    # zero the bottom two halo rows on the r=0 strip (g=-2,-1):
                    # the phase formula extended below the image is invalid there
                    nc.vector.tensor_scalar_mul(up_r[:, 0:2, :], up_r[:, 0:2, :],
                                                mk_t[:, 0:1])
                if fp8r:
                    for dx in need_dx:
                        nc.scalar.copy(
                            out=upf_t[:, dx, 2 * a0:2 * a0 + 12, :],
                            in_=up_r[:, 2 * a0:2 * a0 + 12, dx:dx + W])

            # ---- stage B: effective-cell conv -> out ----
            if fp8r:
                _stage_b_fp8r(nc, tc, up_r, upf_t, wb_t, wr_t, bigs, pairs,
                              psB, psR, outp, out_d)
            elif hyb:
                with (
                    tc.tile_pool(name="vyp", bufs=2) as vyp,
                    tc.tile_pool(name="smp", bufs=2) as smp,
                ):
                    # per-tap (row, col) bases into vector-engine registers
                    rvs = [nc.vector.value_load(co_t[0:1, k, 0:1],
                                                min_val=0, max_val=3)
                           for k in range(9)]
                    cvs = [nc.vector.value_load(co_t[0:1, k, 1:2],
                                                min_val=0, max_val=3)
                           for k in range(9)]
                    mm = mybir.AluOpType.mult
                    aa = mybir.AluOpType.add
                    up_fl = up_full[:]
                    for sb in range(OUT_R // SBR):
                        vys, samps = [], []
                        for k in range(9):
                            vy = vyp.tile([COUT, SBR, UC], mybir.dt.bfloat16,
                                          tag=f"vy{k}")
                            # [SBR rows x UC cols] shifted window == contiguous
                            # flat block of SBR*UC elements
                            base = rvs[k] * UC + cvs[k] + (SBR * sb) * UC
                            i0 = up_fl[:, bass.ds(base, SBR * UC)].rearrange(
                                "p (a b) -> p a b", b=UC)
                            i1 = up_fl[:, bass.ds(base + UC, SBR * UC)].rearrange(
                                "p (a b) -> p a b", b=UC)
                            nc.vector.tensor_scalar_mul(vy[:], i0, bs_t[:, k, 0:1])
                            nc.vector.scalar_tensor_tensor(
                                out=vy[:], in0=i1, scalar=bs_t[:, k, 1:2],
                                in1=vy[:], op0=mm, op1=aa)
                            vys.append(vy)
                        for k in range(S):
                            sa = smp.tile([COUT, SBR, W], mybir.dt.bfloat16,
                                          tag=f"sa{k}")
                            nc.vector.tensor_scalar_mul(
                                sa[:], vys[k][:, :, 0:W], dsc_t[:, k, 0:1])
                            nc.vector.scalar_tensor_tensor(
                                out=sa[:], in0=vys[k][:, :, 1:W + 1],
                                scalar=dsc_t[:, k, 1:2], in1=sa[:],
                                op0=mm, op1=aa)
                            samps.append(sa)
                        for sub in range(SBR // RBLK):
                            rs = slice(RBLK * sub, RBLK * (sub + 1))
                            bi = (SBR * sb) // RBLK + sub
                            for half in range(2):
                                ps = psB.tile([COUT, RBLK, W], mybir.dt.float32,
                                              tag="psB")
                                nmm = S + 2 * (9 - S)
                                si = 0
                                for k in range(9):
                                    if k < S:
                                        rhss = [samps[k][:, rs, :]]
                                    else:
                                        rhss = [vys[k][:, rs, 0:W],
                                                vys[k][:, rs, 1:W + 1]]
                                    for rhs in rhss:
                                        nc.tensor.matmul(
                                            ps[:], lhsT=wb_t[:, si, half, :],
                                            rhs=rhs, start=(si == 0),
                                            stop=(si == nmm - 1))
                                        si += 1
                                ob = outp.tile([COUT, RBLK, W], mybir.dt.float32,
                                               tag="ob")
                                nc.scalar.copy(out=ob[:], in_=ps[:])
                                nc.sync.dma_start(
                                    out=out_d[128 * half:128 * (half + 1),
                                              RBLK * bi:RBLK * (bi + 1), :],
                                    in_=ob[:])
            else:
                if dyn:
                    # per-slot (row, col) bases into tensor-engine registers
                    rvs = [nc.tensor.value_load(co_t[0:1, ci, 0:1],
                                                min_val=0, max_val=4)
                           for ci in range(ncell)]
                    cvs = [nc.tensor.value_load(co_t[0:1, ci, 1:2],
                                                min_val=0, max_val=4)
                           for ci in range(ncell)]
                for bi in range(OUT_R // RBLK):
                    for half in range(2):
                        ps = psB.tile([COUT, RBLK, W], mybir.dt.float32, tag="psB")
                        for ci in range(ncell):
                            if dyn:
                                rhs = up_r[:, bass.ds(rvs[ci] + 4 * bi, RBLK),
                                           bass.ds(cvs[ci], W)]
                            else:
                                dyi, dxi = cells[ci] // 5, cells[ci] % 5
                                ys = 4 * bi + dyi  # up row = o_l + 2 + (dyi-2)
                                rhs = up_r[:, ys:ys + RBLK, dxi:dxi + W]
                            nc.tensor.matmul(
                                ps[:],
                                lhsT=wb_t[:, ci, half, :],
                                rhs=rhs,
                                start=(ci == 0), stop=(ci == ncell - 1),
                            )
                        ob = outp.tile([COUT, RBLK, W], mybir.dt.float32, tag="ob")
                        nc.scalar.copy(out=ob[:], in_=ps[:])
                        nc.sync.dma_start(
                            out=out_d[128 * half:128 * (half + 1),
                                      4 * bi:4 * bi + RBLK, :],
                            in_=ob[:],
                        )

    nc.compile()
    return nc


def _stage_b_fp8r(nc, tc, up_r, upf_t, wb_t, wr_t, bigs, pairs,
                  psB, psR, outp, out_d):
    """Stage B with big cells in bf16 and ring-cell pairs in fp8 DoubleRow.

    upf_t[dx] holds a margin-free fp8 copy of up cols [dx, dx+128), so every
    cell window is a contiguous 512-element block and pair steps are
    automatically 16-aligned (multiples of 128)."""
    mm = mybir.AluOpType.mult
    aa = mybir.AluOpType.add

    # (fp8 casts of up are emitted inline with stage A, band by band)

    upf_fl = upf_t[:].rearrange("p a b c -> p (a b c)")

    def cell_off(c, bi):
        return (c % 5) * (UR * W) + ((4 * bi) + (c // 5)) * W

    G = 2  # blocks per weight-reuse group
    for half in range(2):
        for bg in range(OUT_R // RBLK // G):
            pscs = [psB.tile([COUT, RBLK, W], mybir.dt.float32, tag="psB",
                             name=f"psc_{half}_{bg}_{g}") for g in range(G)]
            for si, ci in enumerate(bigs):
                dyi, dxi = ci // 5, ci % 5
                for g in range(G):
                    bi = G * bg + g
                    ys = 4 * bi + dyi
                    nc.tensor.matmul(
                        pscs[g][:], lhsT=wb_t[:, si, half, :],
                        rhs=up_r[:, ys:ys + RBLK, dxi:dxi + W],
                        start=(si == 0), stop=(si == len(bigs) - 1))
            psrs = None
            if pairs:
                psrs = [psR.tile([COUT, RBLK, W], mybir.dt.float32, tag="psR",
                                 name=f"psr_{half}_{bg}_{g}") for g in range(G)]
                for p, (c1, c2) in enumerate(pairs):
                    step = cell_off(c2, 0) - cell_off(c1, 0)
                    assert step > 0 and step % 16 == 0
                    for g in range(G):
                        bi = G * bg + g
                        win = upf_fl[:, cell_off(c1, bi):cell_off(c1, bi) + RBLK * W]
                        rhs = bass.AP(tensor=win.tensor, offset=win.offset,
                                      ap=[win.ap[0], [step, 2], win.ap[1]])
                        nc.tensor.matmul(
                            psrs[g][:], lhsT=wr_t[:, p, half, :, :], rhs=rhs,
                            perf_mode=mybir.MatmulPerfMode.DoubleRow,
                            start=(p == 0), stop=(p == len(pairs) - 1))
            for g in range(G):
                bi = G * bg + g
                ob = outp.tile([COUT, RBLK, W], mybir.dt.float32, tag="ob")
                nc.scalar.copy(out=ob[:], in_=pscs[g][:])
                if pairs:
                    # TensorScalarPtr may read only one PSUM input
                    nc.vector.scalar_tensor_tensor(
                        out=ob[:], in0=psrs[g][:], scalar=1.0 / RING_SCALE,
                        in1=ob[:], op0=mm, op1=aa)
                nc.sync.dma_start(
                    out=out_d[128 * half:128 * (half + 1),
                              RBLK * bi:RBLK * (bi + 1), :],
                    in_=ob[:])


# --------------------------------------------------------------------------
# entry point
# --------------------------------------------------------------------------

def kernel(x, lateral_feat, trans_w, off_w1, off_b1, off_w2, off_b2):
    x = np.asarray(x)
    oy, ox = _offsets_from_inputs(np.asarray(lateral_feat), np.asarray(off_w1),
                                  np.asarray(off_b1), np.asarray(off_w2),
                                  np.asarray(off_b2))
    in_maps, ncell = _prep_in_maps(x, np.asarray(trans_w), oy, ox)

    key = (VARIANT, ncell)
    if key not in _CACHED_NC:
        _CACHED_NC[key] = _build_nc(ncell)
    nc = _CACHED_NC[key]

    res = run_bass_kernel_spmd(nc, in_maps, core_ids=list(range(N_CORES)))

    out = np.empty((N_BATCH, CIN, H, W), np.float32)
    for core in range(N_CORES):
        n, r = core // STRIPS, core % STRIPS
        out[n, :, OUT_R * r:OUT_R * (r + 1), :] = res.results[core]["out"]
    return out

